# revision 9
# baseline (speedup 1.0000x reference)
import os

import numpy as np
import ml_dtypes

B = 512
H = W = 112
HW = H * W
NCORES = 8
BPC = B // NCORES
P = 128
HALF = HW // 2
QROW = HW // 4
SLOTS = 32
CAP = NCORES * SLOTS

CHUNKS_MASK = [1344, 1120, 672]
assert sum(CHUNKS_MASK) == QROW
CHUNKS_FULL = [1792, 1792, 1568, 1120]
assert sum(CHUNKS_FULL) == HALF

AMR_COLS = 96
NBLK = 12
SHIP = os.environ.get("KERNEL_SHIP", "bf16")

_NC_CACHE = {}


def _build_nc(masked):
    import concourse.bacc as bacc
    import concourse.tile as tile
    from concourse import mybir

    import bass_rust
    from concourse.hw_specs import get_activation_tables

    f32 = mybir.dt.float32
    bf16 = mybir.dt.bfloat16
    fp8 = mybir.dt.float8e4
    AF = mybir.ActivationFunctionType
    OP = mybir.AluOpType
    AX = mybir.AxisListType

    chunks = CHUNKS_MASK if masked else CHUNKS_FULL
    row = QROW if masked else HALF
    nchunk = len(chunks)

    nc = bacc.Bacc("TRN2", target_bir_lowering=False, debug=False,
                   num_devices=NCORES)
    act_set_id = list(get_activation_tables("gen3").keys()).index(
        "natural_log_exp_and_others")

    ship_dt = fp8 if SHIP == "fp8" else bf16
    abc = nc.dram_tensor("abc", [P, 3 * row], ship_dt,
                         kind="ExternalInput").ap()
    small = nc.dram_tensor("small", [P, 2 * NBLK], f32,
                           kind="ExternalInput").ap()
    outp = nc.dram_tensor("out", [1, 1], f32, kind="ExternalOutput").ap()

    with tile.TileContext(nc) as tc:
        with (
            tc.tile_pool(name="big", bufs=nchunk) as big,
            tc.tile_pool(name="jk", bufs=2) as jk,
            tc.tile_pool(name="sm", bufs=1) as sm,
            tc.tile_pool(name="ps", bufs=1, space="PSUM") as ps,
        ):
            nc.scalar.add_instruction(bass_rust.InstLoadActFuncSet(
                name=nc.get_next_instruction_name(),
                engine=mybir.EngineType.Activation,
                act_func_set_id=act_set_id,
            ))

            smt = sm.tile([P, 2 * NBLK], f32)
            nc.sync.dma_start(out=smt, in_=small)
            ones = sm.tile([P, 1], f32)
            nc.vector.memset(ones, 1.0)

            abct = []
            off = 0
            for ci, cf in enumerate(chunks):
                t = big.tile([P, 3 * cf], bf16, tag="abct")
                if SHIP == "fp8":
                    nc.gpsimd.dma_start(out=t,
                                        in_=abc[:, 3 * off:3 * (off + cf)])
                else:
                    eng = nc.sync if ci % 2 == 0 else nc.scalar
                    eng.dma_start(out=t, in_=abc[:, 3 * off:3 * (off + cf)])
                abct.append(t)
                off += cf

            r = smt[:, 0:2 * NBLK].rearrange("p (b c) -> p b c", b=2)
            X0 = r[:, :, 0:6:2]
            X1 = r[:, :, 1:6:2]
            YC = r[:, :, 6:9]
            COND = r[:, :, 9:10]

            Dt = sm.tile([P, 6], f32)
            Dr = Dt[:, 0:6].rearrange("p (b c) -> p b c", b=2)
            nc.vector.tensor_sub(Dr, X1, X0)
            SDt = sm.tile([P, 6], f32)
            SDr = SDt[:, 0:6].rearrange("p (b c) -> p b c", b=2)
            nc.vector.tensor_mul(SDr, Dr, YC)
            Et = sm.tile([P, 6], f32)
            Er = Et[:, 0:6].rearrange("p (b c) -> p b c", b=2)
            nc.scalar.activation(out=Er, in_=SDr, func=AF.Exp)
            CE3 = sm.tile([P, 6], f32)
            CE3r = CE3[:, 0:6].rearrange("p (b c) -> p b c", b=2)
            nc.scalar.activation(out=CE3r, in_=Er, func=AF.Ln, bias=1.0)

            tv = sm.tile([P, 2], f32)
            tvr = tv[:, 0:2].rearrange("p (b c) -> p b c", b=2)
            nc.vector.tensor_mul(tvr, COND, Er[:, :, 0:1])
            nc.vector.tensor_scalar_add(tv, tv, 1.0)
            wv = sm.tile([P, 2], f32)
            nc.vector.reciprocal(wv, tv)

            q = sm.tile([P, 1], f32)
            nc.vector.reduce_sum(q, CE3[:, 0:3], axis=AX.X)
            cepart = sm.tile([P, 1], f32)
            nc.vector.scalar_tensor_tensor(out=cepart, in0=q,
                                           scalar=1.0 / (4 * B), in1=wv[:, 0:1],
                                           op0=OP.mult, op1=OP.mult)
            coef_er = sm.tile([P, 1], f32)
            nc.vector.scalar_tensor_tensor(out=coef_er, in0=wv[:, 1:2],
                                           scalar=1.0 / (B * HW),
                                           in1=smt[:, NBLK + 10:NBLK + 11],
                                           op0=OP.mult, op1=OP.mult)
            coef_sp = smt[:, NBLK + 11:NBLK + 12]

            acc = sm.tile([P, 4 * nchunk], f32)
            for ci, cf in enumerate(chunks):
                t = abct[ci]
                at = t[:, 0:cf]
                bt = t[:, cf:2 * cf]
                ct = t[:, 2 * cf:3 * cf]
                ka = cf - AMR_COLS
                d = big.tile([P, cf], bf16, tag="d")
                nc.vector.tensor_sub(d, at, bt)
                jd = jk.tile([P, cf], bf16, tag="junk")
                nc.scalar.activation(out=jd[:, 0:ka], in_=d[:, 0:ka],
                                     func=AF.Square,
                                     accum_out=acc[:, 2 * ci:2 * ci + 1])
                nc.vector.affine_mul_reduce(
                    out=jd[:, ka:cf], accum_out=acc[:, 2 * ci + 1:2 * ci + 2],
                    in0=d[:, ka:cf], in1=d[:, ka:cf], scale=1.0, bias=0.0)
                e = big.tile([P, cf], bf16, tag="e")
                nc.vector.tensor_sub(e, at, ct)
                je = jk.tile([P, cf], bf16, tag="junk")
                so = 2 * nchunk
                nc.scalar.activation(out=je[:, 0:ka], in_=e[:, 0:ka],
                                     func=AF.Square,
                                     accum_out=acc[:, so + 2 * ci:so + 2 * ci + 1])
                nc.vector.affine_mul_reduce(
                    out=je[:, ka:cf],
                    accum_out=acc[:, so + 2 * ci + 1:so + 2 * ci + 2],
                    in0=e[:, ka:cf], in1=e[:, ka:cf], scale=1.0, bias=0.0)

            pt = ps.tile([1, 4 * nchunk + 1], f32)
            n2 = 2 * nchunk
            nc.tensor.matmul(out=pt[:, 0:n2], lhsT=coef_er, rhs=acc[:, 0:n2],
                             start=True, stop=True)
            nc.tensor.matmul(out=pt[:, n2:2 * n2], lhsT=coef_sp,
                             rhs=acc[:, n2:2 * n2], start=True, stop=True)
            nc.tensor.matmul(out=pt[:, 2 * n2:2 * n2 + 1], lhsT=cepart,
                             rhs=ones, start=True, stop=True)

            res_sb = sm.tile([1, 1], f32)
            nc.vector.reduce_sum(res_sb, pt[:, 0:2 * n2 + 1], axis=AX.X)
            nc.sync.dma_start(out=outp, in_=res_sb)

    nc.compile()
    return nc


def _get_nc(masked):
    key = "mask" if masked else "full"
    if key not in _NC_CACHE:
        _NC_CACHE[key] = _build_nc(masked)
    return _NC_CACHE[key]


def _interleave_ship(a, b, c, chunks):
    row = a.shape[1]
    abc = np.empty((P, 3 * row), dtype=np.float32)
    off = 0
    for cf in chunks:
        sl = slice(off, off + cf)
        abc[:, 3 * off:3 * off + cf] = a[:, sl]
        abc[:, 3 * off + cf:3 * off + 2 * cf] = b[:, sl]
        abc[:, 3 * off + 2 * cf:3 * off + 3 * cf] = c[:, sl]
        off += cf
    dt = ml_dtypes.float8_e4m3 if SHIP == "fp8" else ml_dtypes.bfloat16
    return abc.astype(dt)


def _small_block(p1i, p1o, p2i, pbi, y):
    n = len(y)
    blk = np.zeros((n, NBLK), dtype=np.float32)
    yf = y.astype(np.float32)
    blk[:, 0:2] = p1i
    blk[:, 2:4] = p2i
    pb = pbi.copy()
    pb[y == 0] = np.array([0.0, -100.0], dtype=np.float32)
    blk[:, 4:6] = pb
    blk[:, 6] = 1.0 - 2.0 * yf
    blk[:, 7] = 1.0 - 2.0 * yf
    blk[:, 8] = 1.0
    cur = (p1i[:, 1] > p1i[:, 0])
    flag = (p1o[:, 1] > p1o[:, 0])
    cond = (cur != flag) & (~cur) & (y == 1)
    blk[:, 9] = cond.astype(np.float32)
    same = (cur == flag).astype(np.float32)
    blk[:, 10] = yf
    blk[:, 11] = yf * same / (B * HW)
    return blk


def _small_cam_block(p1i, p1o, y, repl):
    n = len(y)
    blk = np.zeros((n, NBLK), dtype=np.float32)
    yf = y.astype(np.float32)
    blk[:, 0:2] = p1i
    blk[:, 6] = -1.0
    cur = (p1i[:, 1] > p1i[:, 0])
    flag = (p1o[:, 1] > p1o[:, 0])
    cond = (cur != flag) & (~cur) & (y == 1)
    blk[:, 9] = cond.astype(np.float32)
    same = (cur == flag).astype(np.float32)
    blk[:, 10] = yf
    blk[:, 11] = yf * same / (B * HW)
    return np.repeat(blk, repl, axis=0)


def kernel(preds1, cams1, preds1_back, preds2, cams2, y, index):
    from concourse.bass_utils import run_bass_kernel_spmd

    idx = int(np.asarray(index))
    preds1 = np.asarray(preds1, dtype=np.float32)
    preds1_back = np.asarray(preds1_back, dtype=np.float32)
    preds2 = np.asarray(preds2, dtype=np.float32)
    cams1 = np.asarray(cams1, dtype=np.float32)
    cams2 = np.asarray(cams2, dtype=np.float32)
    yi = np.asarray(y).astype(np.int64).reshape(B)

    sel = np.flatnonzero(yi == 1)
    masked = len(sel) <= CAP
    nc = _get_nc(masked)

    chunks = CHUNKS_MASK if masked else CHUNKS_FULL
    row = QROW if masked else HALF
    slots = SLOTS if masked else BPC
    repl = 4 if masked else 2

    in_maps = []
    for k in range(NCORES):
        s = slice(k * BPC, (k + 1) * BPC)
        ce_blk = _small_block(preds1[idx, s], preds1[1 - idx, s],
                              preds2[idx, s], preds1_back[idx, s], yi[s])
        ce_blk = np.repeat(ce_blk, 2, axis=0)

        if masked:
            sel_k = sel[k * SLOTS:(k + 1) * SLOTS]
            nk = len(sel_k)
            a = np.zeros((slots, HW), dtype=np.float32)
            b = np.zeros((slots, HW), dtype=np.float32)
            c = np.zeros((slots, HW), dtype=np.float32)
            a[:nk] = cams1[idx, sel_k, 1].reshape(nk, HW)
            b[:nk] = cams2[idx, sel_k, 1].reshape(nk, HW)
            c[:nk] = cams1[1 - idx, sel_k, 1].reshape(nk, HW)
            p1i = np.zeros((slots, 2), dtype=np.float32)
            p1o = np.zeros((slots, 2), dtype=np.float32)
            ys = np.zeros(slots, dtype=np.int64)
            p1i[:nk] = preds1[idx, sel_k]
            p1o[:nk] = preds1[1 - idx, sel_k]
            ys[:nk] = yi[sel_k]
            cam_blk = _small_cam_block(p1i, p1o, ys, repl)
        else:
            a = cams1[idx, s, 1].reshape(BPC, HW)
            b = cams2[idx, s, 1].reshape(BPC, HW)
            c = cams1[1 - idx, s, 1].reshape(BPC, HW)
            cam_blk = _small_cam_block(preds1[idx, s], preds1[1 - idx, s],
                                       yi[s], repl)

        im = {
            "abc": _interleave_ship(a.reshape(P, row), b.reshape(P, row),
                                    c.reshape(P, row), chunks),
            "small": np.ascontiguousarray(
                np.concatenate([ce_blk, cam_blk], axis=1)),
        }
        in_maps.append(im)

    trace = bool(int(os.environ.get("KERNEL_TRACE", "0")))
    res = run_bass_kernel_spmd(nc, in_maps, core_ids=list(range(NCORES)),
                               trace=trace)
    kernel.last_exec_time_ns = res.exec_time_ns
    kernel.last_result = res
    total = sum(float(res.results[k]["out"][0, 0]) for k in range(NCORES))
    return np.array(total, dtype=np.float32)


kernel.last_exec_time_ns = None
kernel.last_result = None


# revision 13
# speedup vs baseline: 1.3523x; 1.3523x over previous
import os

import numpy as np
import ml_dtypes

B = 512
H = W = 112
HW = H * W
NCORES = 8
BPC = B // NCORES
P = 128
HALF = HW // 2
QROW = HW // 4
SLOTS = 32
CAP = NCORES * SLOTS

CHUNKS_MASK = [392, 392, 560, 560, 616, 616]
assert sum(CHUNKS_MASK) == QROW
GROUPS_MASK = [3, 3]
CHUNKS_FULL = [784, 784, 1120, 1120, 1232, 1232]
assert sum(CHUNKS_FULL) == HALF
GROUPS_FULL = [3, 3]

ACT_FRAC = 0.72
PE_FRAC = 0.12
NBLK = 12
SHIP = os.environ.get("KERNEL_SHIP", "bf16")

_NC_CACHE = {}


def _build_nc(masked):
    import concourse.bacc as bacc
    import concourse.tile as tile
    from concourse import mybir

    import bass_rust
    from concourse.hw_specs import get_activation_tables

    f32 = mybir.dt.float32
    bf16 = mybir.dt.bfloat16
    fp8 = mybir.dt.float8e4
    AF = mybir.ActivationFunctionType
    OP = mybir.AluOpType
    AX = mybir.AxisListType

    chunks = CHUNKS_MASK if masked else CHUNKS_FULL
    row = QROW if masked else HALF
    nchunk = len(chunks)

    nc = bacc.Bacc("TRN2", target_bir_lowering=False, debug=False,
                   num_devices=NCORES)
    act_set_id = list(get_activation_tables("gen3").keys()).index(
        "natural_log_exp_and_others")

    groups = GROUPS_MASK if masked else GROUPS_FULL
    ship_dt = fp8 if SHIP == "fp8" else bf16
    abc = nc.dram_tensor("abc", [P, 3 * row], ship_dt,
                         kind="ExternalInput").ap()
    small = nc.dram_tensor("small", [P, 2 * NBLK], f32,
                           kind="ExternalInput").ap()
    outp = nc.dram_tensor("out", [1, 1], f32, kind="ExternalOutput").ap()

    gcols = []
    chunk_group = []
    ci = 0
    for g, ng in enumerate(groups):
        off = 0
        for _ in range(ng):
            chunk_group.append((g, off))
            off += chunks[ci]
            ci += 1
        gcols.append(off)
    ngrp = len(groups)

    with tile.TileContext(nc) as tc:
        with (
            tc.tile_pool(name="big", bufs=nchunk) as big,
            tc.tile_pool(name="grp", bufs=ngrp) as grp,
            tc.tile_pool(name="jk", bufs=2) as jk,
            tc.tile_pool(name="sm", bufs=1) as sm,
            tc.tile_pool(name="ps", bufs=1, space="PSUM") as ps,
        ):
            smt = sm.tile([P, 2 * NBLK], f32)
            nc.scalar.dma_start(out=smt, in_=small)
            nc.scalar.add_instruction(bass_rust.InstLoadActFuncSet(
                name=nc.get_next_instruction_name(),
                engine=mybir.EngineType.Activation,
                act_func_set_id=act_set_id,
            ))
            ones = sm.tile([P, 1], f32)
            nc.vector.memset(ones, 1.0)

            abct = []
            off = 0
            for ci, cf in enumerate(chunks):
                t = big.tile([P, 3 * cf], bf16, tag="abct")
                nc.sync.dma_start(out=t, in_=abc[:, 3 * off:3 * (off + cf)])
                abct.append(t)
                off += cf

            dg = [grp.tile([P, gcols[g]], bf16, tag="dg", name=f"dg{g}")
                  for g in range(ngrp)]
            eg = [grp.tile([P, gcols[g]], bf16, tag="eg", name=f"eg{g}")
                  for g in range(ngrp)]

            r = smt[:, 0:2 * NBLK].rearrange("p (b c) -> p b c", b=2)
            X0 = r[:, :, 0:6:2]
            X1 = r[:, :, 1:6:2]
            YC = r[:, :, 6:9]
            COND = r[:, :, 9:10]

            Dt = sm.tile([P, 6], f32)
            Dr = Dt[:, 0:6].rearrange("p (b c) -> p b c", b=2)
            nc.gpsimd.tensor_sub(Dr, X1, X0)
            SDt = sm.tile([P, 6], f32)
            SDr = SDt[:, 0:6].rearrange("p (b c) -> p b c", b=2)
            nc.gpsimd.tensor_mul(SDr, Dr, YC)
            Et = sm.tile([P, 6], f32)
            Er = Et[:, 0:6].rearrange("p (b c) -> p b c", b=2)
            nc.scalar.activation(out=Er, in_=SDr, func=AF.Exp)
            CE3 = sm.tile([P, 6], f32)
            CE3r = CE3[:, 0:6].rearrange("p (b c) -> p b c", b=2)
            nc.scalar.activation(out=CE3r, in_=Er, func=AF.Ln, bias=1.0)

            tv = sm.tile([P, 2], f32)
            tvr = tv[:, 0:2].rearrange("p (b c) -> p b c", b=2)
            nc.gpsimd.tensor_mul(tvr, COND, Er[:, :, 0:1])
            nc.gpsimd.tensor_scalar_add(tv, tv, 1.0)
            q = sm.tile([P, 1], f32)
            nc.gpsimd.tensor_add(q, CE3[:, 0:1], CE3[:, 1:2])
            nc.gpsimd.tensor_add(q, q, CE3[:, 2:3])

            wv = sm.tile([P, 2], f32)
            nc.vector.reciprocal(wv, tv)
            cepart = sm.tile([P, 1], f32)
            nc.vector.scalar_tensor_tensor(out=cepart, in0=q,
                                           scalar=1.0 / (4 * B), in1=wv[:, 0:1],
                                           op0=OP.mult, op1=OP.mult)
            coef_er = sm.tile([P, 1], f32)
            nc.vector.scalar_tensor_tensor(out=coef_er, in0=wv[:, 1:2],
                                           scalar=1.0 / (B * HW),
                                           in1=smt[:, NBLK + 10:NBLK + 11],
                                           op0=OP.mult, op1=OP.mult)
            coef_sp = smt[:, NBLK + 11:NBLK + 12]

            for ci, cf in enumerate(chunks):
                t = abct[ci]
                g, og = chunk_group[ci]
                at = t[:, 0:cf]
                nc.vector.tensor_sub(dg[g][:, og:og + cf], at,
                                     t[:, cf:2 * cf])
                nc.vector.tensor_sub(eg[g][:, og:og + cf], at,
                                     t[:, 2 * cf:3 * cf])

            acc = sm.tile([P, 4 * ngrp], f32)
            so = 2 * ngrp
            for g in range(ngrp):
                gc = gcols[g]
                xg = int(gc * ACT_FRAC)
                jd = jk.tile([P, gc], bf16, tag="junk")
                nc.scalar.activation(out=jd[:, 0:xg], in_=dg[g][:, 0:xg],
                                     func=AF.Square,
                                     accum_out=acc[:, 2 * g:2 * g + 1])
                je = jk.tile([P, gc], bf16, tag="junk")
                nc.vector.affine_mul_reduce(
                    out=je[:, xg:gc], accum_out=acc[:, 2 * g + 1:2 * g + 2],
                    in0=dg[g][:, xg:gc], in1=dg[g][:, xg:gc],
                    scale=1.0, bias=0.0)
                nc.scalar.activation(out=jd[:, 0:xg], in_=eg[g][:, 0:xg],
                                     func=AF.Square,
                                     accum_out=acc[:, so + 2 * g:so + 2 * g + 1])
                nc.vector.affine_mul_reduce(
                    out=je[:, xg:gc],
                    accum_out=acc[:, so + 2 * g + 1:so + 2 * g + 2],
                    in0=eg[g][:, xg:gc], in1=eg[g][:, xg:gc],
                    scale=1.0, bias=0.0)

            pt = ps.tile([1, 4 * ngrp + 1], f32)
            nc.tensor.matmul(out=pt[:, 0:so], lhsT=coef_er, rhs=acc[:, 0:so],
                             start=True, stop=True)
            nc.tensor.matmul(out=pt[:, so:2 * so], lhsT=coef_sp,
                             rhs=acc[:, so:2 * so], start=True, stop=True)
            nc.tensor.matmul(out=pt[:, 2 * so:2 * so + 1], lhsT=cepart,
                             rhs=ones, start=True, stop=True)

            res_sb = sm.tile([1, 1], f32)
            nc.vector.reduce_sum(res_sb, pt[:, 0:2 * so + 1], axis=AX.X)
            nc.sync.dma_start(out=outp, in_=res_sb)

    nc.compile()
    return nc


def _get_nc(masked):
    key = "mask" if masked else "full"
    if key not in _NC_CACHE:
        _NC_CACHE[key] = _build_nc(masked)
    return _NC_CACHE[key]


def _interleave_ship(a, b, c, chunks):
    row = a.shape[1]
    abc = np.empty((P, 3 * row), dtype=np.float32)
    off = 0
    for cf in chunks:
        sl = slice(off, off + cf)
        abc[:, 3 * off:3 * off + cf] = a[:, sl]
        abc[:, 3 * off + cf:3 * off + 2 * cf] = b[:, sl]
        abc[:, 3 * off + 2 * cf:3 * off + 3 * cf] = c[:, sl]
        off += cf
    dt = ml_dtypes.float8_e4m3 if SHIP == "fp8" else ml_dtypes.bfloat16
    return abc.astype(dt)


def _small_block(p1i, p1o, p2i, pbi, y):
    n = len(y)
    blk = np.zeros((n, NBLK), dtype=np.float32)
    yf = y.astype(np.float32)
    blk[:, 0:2] = p1i
    blk[:, 2:4] = p2i
    pb = pbi.copy()
    pb[y == 0] = np.array([0.0, -100.0], dtype=np.float32)
    blk[:, 4:6] = pb
    blk[:, 6] = 1.0 - 2.0 * yf
    blk[:, 7] = 1.0 - 2.0 * yf
    blk[:, 8] = 1.0
    cur = (p1i[:, 1] > p1i[:, 0])
    flag = (p1o[:, 1] > p1o[:, 0])
    cond = (cur != flag) & (~cur) & (y == 1)
    blk[:, 9] = cond.astype(np.float32)
    same = (cur == flag).astype(np.float32)
    blk[:, 10] = yf
    blk[:, 11] = yf * same / (B * HW)
    return blk


def _small_cam_block(p1i, p1o, y, repl):
    n = len(y)
    blk = np.zeros((n, NBLK), dtype=np.float32)
    yf = y.astype(np.float32)
    blk[:, 0:2] = p1i
    blk[:, 6] = -1.0
    cur = (p1i[:, 1] > p1i[:, 0])
    flag = (p1o[:, 1] > p1o[:, 0])
    cond = (cur != flag) & (~cur) & (y == 1)
    blk[:, 9] = cond.astype(np.float32)
    same = (cur == flag).astype(np.float32)
    blk[:, 10] = yf
    blk[:, 11] = yf * same / (B * HW)
    return np.repeat(blk, repl, axis=0)


def kernel(preds1, cams1, preds1_back, preds2, cams2, y, index):
    from concourse.bass_utils import run_bass_kernel_spmd

    idx = int(np.asarray(index))
    preds1 = np.asarray(preds1, dtype=np.float32)
    preds1_back = np.asarray(preds1_back, dtype=np.float32)
    preds2 = np.asarray(preds2, dtype=np.float32)
    cams1 = np.asarray(cams1, dtype=np.float32)
    cams2 = np.asarray(cams2, dtype=np.float32)
    yi = np.asarray(y).astype(np.int64).reshape(B)

    sel = np.flatnonzero(yi == 1)
    masked = len(sel) <= CAP
    nc = _get_nc(masked)

    chunks = CHUNKS_MASK if masked else CHUNKS_FULL
    row = QROW if masked else HALF
    slots = SLOTS if masked else BPC
    repl = 4 if masked else 2

    in_maps = []
    for k in range(NCORES):
        s = slice(k * BPC, (k + 1) * BPC)
        ce_blk = _small_block(preds1[idx, s], preds1[1 - idx, s],
                              preds2[idx, s], preds1_back[idx, s], yi[s])
        ce_blk = np.repeat(ce_blk, 2, axis=0)

        if masked:
            sel_k = sel[k * SLOTS:(k + 1) * SLOTS]
            nk = len(sel_k)
            a = np.zeros((slots, HW), dtype=np.float32)
            b = np.zeros((slots, HW), dtype=np.float32)
            c = np.zeros((slots, HW), dtype=np.float32)
            a[:nk] = cams1[idx, sel_k, 1].reshape(nk, HW)
            b[:nk] = cams2[idx, sel_k, 1].reshape(nk, HW)
            c[:nk] = cams1[1 - idx, sel_k, 1].reshape(nk, HW)
            p1i = np.zeros((slots, 2), dtype=np.float32)
            p1o = np.zeros((slots, 2), dtype=np.float32)
            ys = np.zeros(slots, dtype=np.int64)
            p1i[:nk] = preds1[idx, sel_k]
            p1o[:nk] = preds1[1 - idx, sel_k]
            ys[:nk] = yi[sel_k]
            cam_blk = _small_cam_block(p1i, p1o, ys, repl)
        else:
            a = cams1[idx, s, 1].reshape(BPC, HW)
            b = cams2[idx, s, 1].reshape(BPC, HW)
            c = cams1[1 - idx, s, 1].reshape(BPC, HW)
            cam_blk = _small_cam_block(preds1[idx, s], preds1[1 - idx, s],
                                       yi[s], repl)

        im = {
            "abc": _interleave_ship(a.reshape(P, row), b.reshape(P, row),
                                    c.reshape(P, row), chunks),
            "small": np.ascontiguousarray(
                np.concatenate([ce_blk, cam_blk], axis=1)),
        }
        in_maps.append(im)

    trace = bool(int(os.environ.get("KERNEL_TRACE", "0")))
    res = run_bass_kernel_spmd(nc, in_maps, core_ids=list(range(NCORES)),
                               trace=trace)
    kernel.last_exec_time_ns = res.exec_time_ns
    kernel.last_result = res
    total = sum(float(res.results[k]["out"][0, 0]) for k in range(NCORES))
    return np.array(total, dtype=np.float32)


kernel.last_exec_time_ns = None
kernel.last_result = None
